# revision 49
# baseline (speedup 1.0000x reference)
"""Dilated block attention + output projection on 8 trn2 cores.

Sharding: core c handles batch b = c//2 and heads h = 4*(c%2) .. +3.
Each core computes the full dilated-attention combine for its 4 (b,h)
pairs and a partial output projection (contraction over its 4 heads'
256 hidden dims).  The host sums the two half-hidden partials per batch
and adds the bias.

Math note: the reference's stabilized-softmax + detached-expsum
reweighting collapses to the unstabilized form
    out[p] = (sum_d exp(S_d) @ V_d  scattered to p) / (sum_d rowsum exp(S_d))
which is what the kernel computes (scores ~ N(0,1), no overflow risk).

Device layout per (b,h), per dilation branch: the host packs ONE blob
[128, W] per branch holding, in SBUF layout:
  - Q^T [64, Ld] duplicated onto both partition halves (matmul rhs for
    both PE row groups),
  - K^T k-tiles parity-split: even k-tiles on partitions 0-63, odd on
    64-127 (so consecutive K=64 QK matmuls land on different PE row
    groups and run concurrently),
  - V k-tile slabs [128, 65] with a ones column (PV matmul with M=65
    gives the exp row-sum on psum row 64 for free).
One DMA per branch.  S^T = matmul(lhsT=K^T[64,128], rhs=Q^T[64,512]) to
PSUM; exp on ScalarE (PSUM->SBUF, scale=0.125 folds 1/sqrt(hd)); PV
accumulates over k-tiles into a [65, 512] psum window; DVE scatter-adds
windows into per-head accumulators [65, 4096]; 1/w via custom-DVE fast
reciprocal + K=1 ones-matmul partition broadcast; o_proj as 4
accumulating K=64 matmuls per M-tile against Wo^T slices.

Matmul operands are bf16 (fp32 matmuls run as two PE passes on trn2);
psum accumulation and the softmax combine stay fp32.  The PE stream is
software-pipelined (QK/exp of group i+1 issued before PV of group i) so
the in-order PE queue never head-of-line blocks on the ScalarE exp.
"""

import ml_dtypes
import numpy as np

BF16_NP = ml_dtypes.bfloat16

B, H, L, HD = 4, 8, 4096, 64
HIDDEN = H * HD
DILS = (1, 2, 4, 8)
BLOCK = 1024
PB = 4  # (b,h) pairs per core
NCORES = 8
LDS = [L // d for d in DILS]  # 4096, 2048, 1024, 512
NKTS = [ld // 128 for ld in LDS]  # 32, 16, 8, 4
# blob widths per branch: Q dup (Ld) + K parity-split (Ld/2) + V slabs (nkt*65)
WS = [ld + ld // 2 + nkt * 65 for ld, nkt in zip(LDS, NKTS)]
BOFFS = [sum(WS[:i]) for i in range(len(WS))]
WSUM = sum(WS)
QCH = 512  # q-chunk (strided-domain positions) per psum window

_PROGRAM = None


def build_program():
    """Build the (SPMD, identical on all cores) Bass program."""
    from contextlib import ExitStack

    import concourse.tile as tile
    from concourse import bacc, mybir

    F32 = mybir.dt.float32
    BF16 = mybir.dt.bfloat16
    nc = bacc.Bacc("TRN2", target_bir_lowering=False, debug=False)

    blob_d = nc.dram_tensor("blob", [PB, 128, WSUM], BF16, kind="ExternalInput")
    # head-pair-stacked Wo^T slices: wot[p, k<64] = head 2p dim k,
    # wot[p, k>=64] = head 2p+1 dim k-64 (K=128 o_proj contraction)
    wot_d = nc.dram_tensor("wot", [PB // 2, 128, HIDDEN], BF16, kind="ExternalInput")
    out_d = nc.dram_tensor("out", [L, HIDDEN], F32, kind="ExternalOutput")

    with tile.TileContext(nc) as tc, ExitStack() as ctx:
        consts = ctx.enter_context(tc.tile_pool(name="consts", bufs=1))
        br_pool = ctx.enter_context(tc.tile_pool(name="br", bufs=1))
        e_pool = ctx.enter_context(tc.tile_pool(name="ep", bufs=5))
        acc_pool = ctx.enter_context(tc.tile_pool(name="accp", bufs=1))
        io_pool = ctx.enter_context(tc.tile_pool(name="iop", bufs=2))
        # st tiles hold 3 k-tiles of scores (one ACTIVATE of N=1536 instead
        # of 1.5 of N=1024 -- the ~470ns per-call ScalarE overhead is the
        # top non-stream cost on the bottleneck engine).  2 bufs x 3 banks
        # keeps the same 6 in-flight score k-tiles as 3 bufs x 2 banks.
        st_psum = ctx.enter_context(tc.tile_pool(name="stp", bufs=2, space="PSUM"))
        pv_psum = ctx.enter_context(tc.tile_pool(name="pvp", bufs=2, space="PSUM"))

        zero_bias = consts.tile([128, 1], F32, tag="zb")
        nc.vector.memset(zero_bias, 0.0)
        ones_row = consts.tile([1, 64], BF16, tag="ones_row")
        nc.vector.memset(ones_row, 1.0)

        wot_sb = consts.tile([128, PB // 2, HIDDEN], BF16, tag="wot")
        nc.sync.dma_start(out=wot_sb, in_=wot_d.rearrange("j r c -> r j c"))

        acc_tiles = [
            acc_pool.tile([65, L], F32, tag=f"acc{j}", bufs=1, name=f"acc{j}")
            for j in range(PB)
        ]
        # o_proj lhsT: head pair p stacked on partitions [0:64] / [64:128]
        oacc_tiles = [
            acc_pool.tile([128, L], BF16, tag=f"oacc{p}", bufs=1, name=f"oacc{p}")
            for p in range(PB // 2)
        ]

        bt_all = {}

        def get_bt(j, di):
            if (j, di) not in bt_all:
                bufs = 2
                bt = br_pool.tile(
                    [128, WS[di]], BF16, tag=f"b{di}", bufs=bufs, name=f"bt{di}"
                )
                nc.sync.dma_start(
                    out=bt, in_=blob_d[j, :, BOFFS[di] : BOFFS[di] + WS[di]]
                )
                bt_all[(j, di)] = bt
            return bt_all[(j, di)]

        for j in range(PB):
            acc = acc_tiles[j]

            # Build the flat job list: one job per (branch, window, k-group).
            jobs = []
            for di, d in enumerate(DILS):
                Ld = LDS[di]
                bs = min(BLOCK, Ld)
                nblk = Ld // bs
                nkt_blk = bs // 128
                for blk in range(nblk):
                    for qc in range(bs // QCH):
                        q0 = blk * bs + qc * QCH
                        kts = list(range(nkt_blk))
                        groups = [kts[x : x + 3] for x in range(0, nkt_blk, 3)]
                        for gi, g in enumerate(groups):
                            jobs.append(
                                dict(
                                    di=di,
                                    d=d,
                                    blk=blk,
                                    nkt_blk=nkt_blk,
                                    q0=q0,
                                    g=g,
                                    first=(gi == 0),
                                    last=(gi == len(groups) - 1),
                                    done0=sum(len(x) for x in groups[:gi]),
                                )
                            )

            # prefetch the first branches
            get_bt(j, 0)
            get_bt(j, 1)

            def emit_qk_exp(job):
                """QK matmuls for the group -> exp to a bf16 E tile."""
                di, q0, g = job["di"], job["q0"], job["g"]
                Ld = LDS[di]
                kbase = Ld
                bt = get_bt(j, di)
                gs = len(g)
                st = st_psum.tile([128, 3, QCH], F32, tag="st", name="st")
                for i, kt in enumerate(g):
                    tg = job["blk"] * job["nkt_blk"] + kt
                    half = tg % 2
                    k0 = kbase + (tg // 2) * 128
                    nc.tensor.matmul(
                        st[:, i, :],
                        bt[half * 64 : (half + 1) * 64, k0 : k0 + 128],
                        bt[half * 64 : (half + 1) * 64, q0 : q0 + QCH],
                        start=True,
                        stop=True,
                    )
                et = e_pool.tile([128, 3, QCH], BF16, tag="et", name="et")
                nc.scalar.activation(
                    et[:, 0:gs, :],
                    st[:, 0:gs, :],
                    mybir.ActivationFunctionType.Exp,
                    bias=zero_bias,
                    scale=0.125,
                )
                job["et"] = et

            def emit_pv(job):
                """PV accumulation for the group; combine if window done."""
                di, d = job["di"], job["d"]
                Ld = LDS[di]
                vbase = Ld + Ld // 2
                bt = get_bt(j, di)
                et = job["et"]
                pv = job["pv"]
                done = job["done0"]
                for i, kt in enumerate(job["g"]):
                    tg = job["blk"] * job["nkt_blk"] + kt
                    nc.tensor.matmul(
                        pv[0:65, :],
                        bt[:, vbase + tg * 65 : vbase + tg * 65 + 65],
                        et[:, i, :],
                        start=(done == 0),
                        stop=(done == job["nkt_blk"] - 1),
                        skip_group_check=True,
                    )
                    done += 1
                if job["last"]:
                    p0 = job["q0"] * d
                    if d == 1:
                        nc.vector.tensor_copy(
                            out=acc[:, p0 : p0 + QCH], in_=pv[0:65, :]
                        )
                    else:
                        dst = acc[:, p0 : p0 + QCH * d : d]
                        nc.vector.tensor_add(out=dst, in0=dst, in1=pv[0:65, :])

            # software pipeline, depth 2: QK/exp of job i, then PV of job
            # i-2, so the in-order PE queue never blocks on the ACT exp.
            from collections import deque

            pending = deque()
            cur_pv = None
            for idx, job in enumerate(jobs):
                if job["first"]:
                    cur_pv = pv_psum.tile([128, QCH], F32, tag="pv", name="pv")
                job["pv"] = cur_pv
                if idx > 0 and job["di"] != jobs[idx - 1]["di"]:
                    # prefetch next branch blob one branch ahead
                    if job["di"] < 3:
                        get_bt(j, job["di"] + 1)
                    # prefetch the NEXT head's big blobs mid-head (bufs=2)
                    # so its first QKs never wait on the 2.1MB d=1 DMA
                    if job["di"] == 2 and j + 1 < PB:
                        get_bt(j + 1, 0)
                        get_bt(j + 1, 1)
                emit_qk_exp(job)
                pending.append(job)
                if len(pending) > 3:
                    emit_pv(pending.popleft())
            while pending:
                emit_pv(pending.popleft())

            # normalize: oacc = acc[0:64, :] * (1 / acc[64, :]) in bf16.
            # The w row [1, L] is reshaped to [128, 32] by DMA so the
            # reciprocal + cast run on all 128 DVE lanes instead of one.
            wrs = io_pool.tile([128, L // 128], F32, tag="wrs", bufs=2)
            nc.sync.dma_start(out=wrs, in_=acc[64:65, :])
            nc.vector.reciprocal_approx_fast(out=wrs, in_=wrs)
            wrsb = io_pool.tile([128, L // 128], BF16, tag="wrsb", bufs=2)
            nc.vector.tensor_copy(out=wrsb, in_=wrs)
            wrowb = io_pool.tile([1, L], BF16, tag="wrowb", bufs=2)
            nc.sync.dma_start(out=wrowb, in_=wrsb)
            half = j % 2
            if half == 0:
                odst = oacc_tiles[j // 2]
            else:
                odst = io_pool.tile([64, L], BF16, tag="oscr", bufs=2)
            for w in range(L // QCH):
                ws = slice(w * QCH, (w + 1) * QCH)
                # broadcast 1/w across 64 partitions via a K=1 ones-matmul
                bc = pv_psum.tile([64, QCH], F32, tag="pv", name="bc")
                nc.tensor.matmul(
                    bc, ones_row[0:1, :], wrowb[0:1, ws], start=True, stop=True
                )
                nc.vector.tensor_mul(
                    out=odst[0:64, ws], in0=acc[0:64, ws], in1=bc
                )
            if half == 1:
                # DVE lanes are partition-locked; a SBUF->SBUF DMA moves the
                # odd head onto partitions 64..127 of the pair-stacked lhsT
                nc.sync.dma_start(out=oacc_tiles[j // 2][64:128, :], in_=odst)

        # partial o_proj: out[p, :] = sum_pairs oaccP[:, p]^T @ wot_p (K=128)
        for mt in range(L // 128):
            po = pv_psum.tile([128, HIDDEN], F32, tag="pv", name="po")
            for p in range(PB // 2):
                nc.tensor.matmul(
                    po,
                    oacc_tiles[p][:, mt * 128 : (mt + 1) * 128],
                    wot_sb[:, p, :],
                    start=(p == 0),
                    stop=(p == PB // 2 - 1),
                    skip_group_check=True,
                )
            ot = io_pool.tile([128, HIDDEN], F32, tag="ot")
            if mt % 2 == 0:
                nc.scalar.copy(out=ot, in_=po)
            else:
                nc.vector.tensor_copy(out=ot, in_=po)
            nc.sync.dma_start(out=out_d[mt * 128 : (mt + 1) * 128, :], in_=ot)

    nc.compile()
    return nc


def get_program():
    global _PROGRAM
    if _PROGRAM is None:
        _PROGRAM = build_program()
    return _PROGRAM


def _branch_blob(qT, kT, vv, di):
    """Pack one dilation branch into the [128, W] SBUF-layout blob.

    qT, kT: [64, Ld] transposed Q/K for this branch; vv: [Ld, 65] V plus
    ones column."""
    Ld, nkt = LDS[di], NKTS[di]
    q_part = np.concatenate([qT, qT], axis=0)  # [128, Ld]
    k3 = kT.reshape(64, nkt, 128)
    k_part = np.concatenate(
        [
            k3[:, 0::2, :].reshape(64, -1),
            k3[:, 1::2, :].reshape(64, -1),
        ],
        axis=0,
    )  # [128, Ld/2]
    v_part = vv.reshape(nkt, 128, 65).transpose(1, 0, 2).reshape(128, nkt * 65)
    return np.concatenate([q_part, k_part, v_part], axis=1)


def make_in_maps(query_states, key_states, value_states, Wo):
    q = np.asarray(query_states, dtype=np.float32)
    k = np.asarray(key_states, dtype=np.float32)
    v = np.asarray(value_states, dtype=np.float32)
    Wo = np.asarray(Wo, dtype=np.float32)

    in_maps = []
    for c in range(NCORES):
        b, hs = c // 2, (c % 2) * PB
        blob = np.empty((PB, 128, WSUM), BF16_NP)
        wot = np.empty((PB // 2, 128, HIDDEN), BF16_NP)
        for j in range(PB):
            h = hs + j
            for di, d in enumerate(DILS):
                Ld = LDS[di]
                vv = np.empty((Ld, 65), np.float32)
                vv[:, 0:64] = v[b, h, ::d, :]
                vv[:, 64] = 1.0
                blob[j, :, BOFFS[di] : BOFFS[di] + WS[di]] = _branch_blob(
                    np.ascontiguousarray(q[b, h, ::d, :].T),
                    np.ascontiguousarray(k[b, h, ::d, :].T),
                    vv,
                    di,
                )
            # head-pair-stacked o_proj weights (K=128 contraction)
            wot[j // 2, (j % 2) * 64 : (j % 2 + 1) * 64, :] = Wo[
                :, h * 64 : (h + 1) * 64
            ].T
        in_maps.append({"blob": blob, "wot": wot})
    return in_maps


def combine_outputs(results, bo):
    bo = np.asarray(bo, dtype=np.float32)
    out = np.empty((B, L, HIDDEN), np.float32)
    for b in range(B):
        out[b] = results[2 * b]["out"] + results[2 * b + 1]["out"] + bo
    return out


def kernel(
    query_states,
    key_states,
    value_states,
    Wo,
    bo,
    _trace=False,
    _tmpdir=None,
    _results=[None],
):
    from concourse.bass_utils import run_bass_kernel_spmd

    nc = get_program()
    in_maps = make_in_maps(query_states, key_states, value_states, Wo)
    res = run_bass_kernel_spmd(
        nc, in_maps, list(range(NCORES)), trace=_trace, tmpdir=_tmpdir
    )
    _results[0] = res
    return combine_outputs(res.results, bo)



# revision 50
# speedup vs baseline: 1.0038x; 1.0038x over previous
"""Dilated block attention + output projection on 8 trn2 cores.

Sharding: core c handles batch b = c//2 and heads h = 4*(c%2) .. +3.
Each core computes the full dilated-attention combine for its 4 (b,h)
pairs and a partial output projection (contraction over its 4 heads'
256 hidden dims).  The host sums the two half-hidden partials per batch
and adds the bias.

Math note: the reference's stabilized-softmax + detached-expsum
reweighting collapses to the unstabilized form
    out[p] = (sum_d exp(S_d) @ V_d  scattered to p) / (sum_d rowsum exp(S_d))
which is what the kernel computes (scores ~ N(0,1), no overflow risk).

Device layout per (b,h), per dilation branch: the host packs ONE blob
[128, W] per branch holding, in SBUF layout:
  - Q^T [64, Ld] duplicated onto both partition halves (matmul rhs for
    both PE row groups),
  - K^T k-tiles parity-split: even k-tiles on partitions 0-63, odd on
    64-127 (so consecutive K=64 QK matmuls land on different PE row
    groups and run concurrently),
  - V k-tile slabs [128, 65] with a ones column (PV matmul with M=65
    gives the exp row-sum on psum row 64 for free).
One DMA per branch.  S^T = matmul(lhsT=K^T[64,128], rhs=Q^T[64,512]) to
PSUM; exp on ScalarE in 3-k-tile batches (N=1536 ACTIVATEs cut the
~470ns per-call overhead on the bottleneck engine by a third); PV
accumulates over k-tiles into a [65, 512] psum window; DVE scatter-adds
windows into per-head accumulators [65, 4096]; 1/w via a [1,L]->[128,32]
DMA reshape + custom-DVE fast reciprocal (128 lanes instead of 1) + K=1
ones-matmul partition broadcast; o_proj as 2 accumulating K=128 matmuls
per M-tile against head-pair-stacked Wo^T slices (odd heads moved to
partitions 64-127 by a SBUF->SBUF DMA), psum->sbuf copies alternating
ScalarE/VectorE.

Matmul operands are bf16 (fp32 matmuls run as two PE passes on trn2);
psum accumulation and the softmax combine stay fp32.  The PE stream is
software-pipelined (QK/exp of group i+1 issued before PV of group i) so
the in-order PE queue never head-of-line blocks on the ScalarE exp.
"""

import ml_dtypes
import numpy as np

BF16_NP = ml_dtypes.bfloat16

B, H, L, HD = 4, 8, 4096, 64
HIDDEN = H * HD
DILS = (1, 2, 4, 8)
BLOCK = 1024
PB = 4  # (b,h) pairs per core
NCORES = 8
LDS = [L // d for d in DILS]  # 4096, 2048, 1024, 512
NKTS = [ld // 128 for ld in LDS]  # 32, 16, 8, 4
# blob widths per branch: Q dup (Ld) + K parity-split (Ld/2) + V slabs (nkt*65)
WS = [ld + ld // 2 + nkt * 65 for ld, nkt in zip(LDS, NKTS)]
BOFFS = [sum(WS[:i]) for i in range(len(WS))]
WSUM = sum(WS)
QCH = 512  # q-chunk (strided-domain positions) per psum window

_PROGRAM = None


def build_program():
    """Build the (SPMD, identical on all cores) Bass program."""
    from contextlib import ExitStack

    import concourse.tile as tile
    from concourse import bacc, mybir

    F32 = mybir.dt.float32
    BF16 = mybir.dt.bfloat16
    nc = bacc.Bacc("TRN2", target_bir_lowering=False, debug=False)

    blob_d = nc.dram_tensor("blob", [PB, 128, WSUM], BF16, kind="ExternalInput")
    # head-pair-stacked Wo^T slices: wot[p, k<64] = head 2p dim k,
    # wot[p, k>=64] = head 2p+1 dim k-64 (K=128 o_proj contraction)
    wot_d = nc.dram_tensor("wot", [PB // 2, 128, HIDDEN], BF16, kind="ExternalInput")
    out_d = nc.dram_tensor("out", [L, HIDDEN], F32, kind="ExternalOutput")

    with tile.TileContext(nc) as tc, ExitStack() as ctx:
        consts = ctx.enter_context(tc.tile_pool(name="consts", bufs=1))
        br_pool = ctx.enter_context(tc.tile_pool(name="br", bufs=1))
        e_pool = ctx.enter_context(tc.tile_pool(name="ep", bufs=5))
        acc_pool = ctx.enter_context(tc.tile_pool(name="accp", bufs=1))
        io_pool = ctx.enter_context(tc.tile_pool(name="iop", bufs=2))
        # st tiles hold 3 k-tiles of scores (one ACTIVATE of N=1536 instead
        # of 1.5 of N=1024 -- the ~470ns per-call ScalarE overhead is the
        # top non-stream cost on the bottleneck engine).  2 bufs x 3 banks
        # keeps the same 6 in-flight score k-tiles as 3 bufs x 2 banks.
        st_psum = ctx.enter_context(tc.tile_pool(name="stp", bufs=2, space="PSUM"))
        pv_psum = ctx.enter_context(tc.tile_pool(name="pvp", bufs=2, space="PSUM"))

        zero_bias = consts.tile([128, 1], F32, tag="zb")
        nc.vector.memset(zero_bias, 0.0)
        ones_row = consts.tile([1, 64], BF16, tag="ones_row")
        nc.vector.memset(ones_row, 1.0)

        wot_sb = consts.tile([128, PB // 2, HIDDEN], BF16, tag="wot")
        nc.sync.dma_start(out=wot_sb, in_=wot_d.rearrange("j r c -> r j c"))

        acc_tiles = [
            acc_pool.tile([65, L], F32, tag=f"acc{j}", bufs=1, name=f"acc{j}")
            for j in range(PB)
        ]
        # o_proj lhsT: head pair p stacked on partitions [0:64] / [64:128]
        oacc_tiles = [
            acc_pool.tile([128, L], BF16, tag=f"oacc{p}", bufs=1, name=f"oacc{p}")
            for p in range(PB // 2)
        ]

        bt_all = {}

        def get_bt(j, di):
            if (j, di) not in bt_all:
                bufs = 1 if di <= 1 else 2
                bt = br_pool.tile(
                    [128, WS[di]], BF16, tag=f"b{di}", bufs=bufs, name=f"bt{di}"
                )
                nc.sync.dma_start(
                    out=bt, in_=blob_d[j, :, BOFFS[di] : BOFFS[di] + WS[di]]
                )
                bt_all[(j, di)] = bt
            return bt_all[(j, di)]

        for j in range(PB):
            acc = acc_tiles[j]

            # Build the flat job list: one job per (branch, window, k-group).
            jobs = []
            for di, d in enumerate(DILS):
                Ld = LDS[di]
                bs = min(BLOCK, Ld)
                nblk = Ld // bs
                nkt_blk = bs // 128
                for blk in range(nblk):
                    for qc in range(bs // QCH):
                        q0 = blk * bs + qc * QCH
                        kts = list(range(nkt_blk))
                        groups = [kts[x : x + 3] for x in range(0, nkt_blk, 3)]
                        for gi, g in enumerate(groups):
                            jobs.append(
                                dict(
                                    di=di,
                                    d=d,
                                    blk=blk,
                                    nkt_blk=nkt_blk,
                                    q0=q0,
                                    g=g,
                                    first=(gi == 0),
                                    last=(gi == len(groups) - 1),
                                    done0=sum(len(x) for x in groups[:gi]),
                                )
                            )

            # prefetch the first branches
            get_bt(j, 0)
            get_bt(j, 1)

            def emit_qk_exp(job):
                """QK matmuls for the group -> exp to a bf16 E tile."""
                di, q0, g = job["di"], job["q0"], job["g"]
                Ld = LDS[di]
                kbase = Ld
                bt = get_bt(j, di)
                gs = len(g)
                st = st_psum.tile([128, 3, QCH], F32, tag="st", name="st")
                for i, kt in enumerate(g):
                    tg = job["blk"] * job["nkt_blk"] + kt
                    half = tg % 2
                    k0 = kbase + (tg // 2) * 128
                    nc.tensor.matmul(
                        st[:, i, :],
                        bt[half * 64 : (half + 1) * 64, k0 : k0 + 128],
                        bt[half * 64 : (half + 1) * 64, q0 : q0 + QCH],
                        start=True,
                        stop=True,
                    )
                et = e_pool.tile([128, 3, QCH], BF16, tag="et", name="et")
                nc.scalar.activation(
                    et[:, 0:gs, :],
                    st[:, 0:gs, :],
                    mybir.ActivationFunctionType.Exp,
                    bias=zero_bias,
                    scale=0.125,
                )
                job["et"] = et

            def emit_pv(job):
                """PV accumulation for the group; combine if window done."""
                di, d = job["di"], job["d"]
                Ld = LDS[di]
                vbase = Ld + Ld // 2
                bt = get_bt(j, di)
                et = job["et"]
                pv = job["pv"]
                done = job["done0"]
                for i, kt in enumerate(job["g"]):
                    tg = job["blk"] * job["nkt_blk"] + kt
                    nc.tensor.matmul(
                        pv[0:65, :],
                        bt[:, vbase + tg * 65 : vbase + tg * 65 + 65],
                        et[:, i, :],
                        start=(done == 0),
                        stop=(done == job["nkt_blk"] - 1),
                        skip_group_check=True,
                    )
                    done += 1
                if job["last"]:
                    p0 = job["q0"] * d
                    if d == 1:
                        nc.vector.tensor_copy(
                            out=acc[:, p0 : p0 + QCH], in_=pv[0:65, :]
                        )
                    else:
                        dst = acc[:, p0 : p0 + QCH * d : d]
                        nc.vector.tensor_add(out=dst, in0=dst, in1=pv[0:65, :])

            # software pipeline, depth 2: QK/exp of job i, then PV of job
            # i-2, so the in-order PE queue never blocks on the ACT exp.
            from collections import deque

            pending = deque()
            cur_pv = None
            for idx, job in enumerate(jobs):
                if job["first"]:
                    cur_pv = pv_psum.tile([128, QCH], F32, tag="pv", name="pv")
                job["pv"] = cur_pv
                # prefetch next branch blob one branch ahead
                if idx > 0 and job["di"] != jobs[idx - 1]["di"] and job["di"] < 3:
                    get_bt(j, job["di"] + 1)
                emit_qk_exp(job)
                pending.append(job)
                if len(pending) > 3:
                    emit_pv(pending.popleft())
            while pending:
                emit_pv(pending.popleft())

            # normalize: oacc = acc[0:64, :] * (1 / acc[64, :]) in bf16.
            # The w row [1, L] is reshaped to [128, 32] by DMA so the
            # reciprocal + cast run on all 128 DVE lanes instead of one.
            wrs = io_pool.tile([128, L // 128], F32, tag="wrs", bufs=2)
            nc.sync.dma_start(out=wrs, in_=acc[64:65, :])
            nc.vector.reciprocal_approx_fast(out=wrs, in_=wrs)
            wrsb = io_pool.tile([128, L // 128], BF16, tag="wrsb", bufs=2)
            nc.vector.tensor_copy(out=wrsb, in_=wrs)
            wrowb = io_pool.tile([1, L], BF16, tag="wrowb", bufs=2)
            nc.sync.dma_start(out=wrowb, in_=wrsb)
            half = j % 2
            if half == 0:
                odst = oacc_tiles[j // 2]
            else:
                odst = io_pool.tile([64, L], BF16, tag="oscr", bufs=2)
            for w in range(L // QCH):
                ws = slice(w * QCH, (w + 1) * QCH)
                # broadcast 1/w across 64 partitions via a K=1 ones-matmul
                bc = pv_psum.tile([64, QCH], F32, tag="pv", name="bc")
                nc.tensor.matmul(
                    bc, ones_row[0:1, :], wrowb[0:1, ws], start=True, stop=True
                )
                nc.vector.tensor_mul(
                    out=odst[0:64, ws], in0=acc[0:64, ws], in1=bc
                )
            if half == 1:
                # DVE lanes are partition-locked; a SBUF->SBUF DMA moves the
                # odd head onto partitions 64..127 of the pair-stacked lhsT
                nc.sync.dma_start(out=oacc_tiles[j // 2][64:128, :], in_=odst)

        # partial o_proj: out[p, :] = sum_pairs oaccP[:, p]^T @ wot_p (K=128)
        for mt in range(L // 128):
            po = pv_psum.tile([128, HIDDEN], F32, tag="pv", name="po")
            for p in range(PB // 2):
                nc.tensor.matmul(
                    po,
                    oacc_tiles[p][:, mt * 128 : (mt + 1) * 128],
                    wot_sb[:, p, :],
                    start=(p == 0),
                    stop=(p == PB // 2 - 1),
                    skip_group_check=True,
                )
            ot = io_pool.tile([128, HIDDEN], F32, tag="ot")
            if mt % 2 == 0:
                nc.scalar.copy(out=ot, in_=po)
            else:
                nc.vector.tensor_copy(out=ot, in_=po)
            nc.sync.dma_start(out=out_d[mt * 128 : (mt + 1) * 128, :], in_=ot)

    nc.compile()
    return nc


def get_program():
    global _PROGRAM
    if _PROGRAM is None:
        _PROGRAM = build_program()
    return _PROGRAM


def _branch_blob(qT, kT, vv, di):
    """Pack one dilation branch into the [128, W] SBUF-layout blob.

    qT, kT: [64, Ld] transposed Q/K for this branch; vv: [Ld, 65] V plus
    ones column."""
    Ld, nkt = LDS[di], NKTS[di]
    q_part = np.concatenate([qT, qT], axis=0)  # [128, Ld]
    k3 = kT.reshape(64, nkt, 128)
    k_part = np.concatenate(
        [
            k3[:, 0::2, :].reshape(64, -1),
            k3[:, 1::2, :].reshape(64, -1),
        ],
        axis=0,
    )  # [128, Ld/2]
    v_part = vv.reshape(nkt, 128, 65).transpose(1, 0, 2).reshape(128, nkt * 65)
    return np.concatenate([q_part, k_part, v_part], axis=1)


def make_in_maps(query_states, key_states, value_states, Wo):
    q = np.asarray(query_states, dtype=np.float32)
    k = np.asarray(key_states, dtype=np.float32)
    v = np.asarray(value_states, dtype=np.float32)
    Wo = np.asarray(Wo, dtype=np.float32)

    in_maps = []
    for c in range(NCORES):
        b, hs = c // 2, (c % 2) * PB
        blob = np.empty((PB, 128, WSUM), BF16_NP)
        wot = np.empty((PB // 2, 128, HIDDEN), BF16_NP)
        for j in range(PB):
            h = hs + j
            for di, d in enumerate(DILS):
                Ld = LDS[di]
                vv = np.empty((Ld, 65), np.float32)
                vv[:, 0:64] = v[b, h, ::d, :]
                vv[:, 64] = 1.0
                blob[j, :, BOFFS[di] : BOFFS[di] + WS[di]] = _branch_blob(
                    np.ascontiguousarray(q[b, h, ::d, :].T),
                    np.ascontiguousarray(k[b, h, ::d, :].T),
                    vv,
                    di,
                )
            # head-pair-stacked o_proj weights (K=128 contraction)
            wot[j // 2, (j % 2) * 64 : (j % 2 + 1) * 64, :] = Wo[
                :, h * 64 : (h + 1) * 64
            ].T
        in_maps.append({"blob": blob, "wot": wot})
    return in_maps


def combine_outputs(results, bo):
    bo = np.asarray(bo, dtype=np.float32)
    out = np.empty((B, L, HIDDEN), np.float32)
    for b in range(B):
        out[b] = results[2 * b]["out"] + results[2 * b + 1]["out"] + bo
    return out


def kernel(
    query_states,
    key_states,
    value_states,
    Wo,
    bo,
    _trace=False,
    _tmpdir=None,
    _results=[None],
):
    from concourse.bass_utils import run_bass_kernel_spmd

    nc = get_program()
    in_maps = make_in_maps(query_states, key_states, value_states, Wo)
    res = run_bass_kernel_spmd(
        nc, in_maps, list(range(NCORES)), trace=_trace, tmpdir=_tmpdir
    )
    _results[0] = res
    return combine_outputs(res.results, bo)



# revision 54
# speedup vs baseline: 1.1025x; 1.0984x over previous
"""Dilated block attention + output projection on 8 trn2 cores.

Sharding: core c handles batch b = c//2 and heads h = 4*(c%2) .. +3.
Each core computes the full dilated-attention combine for its 4 (b,h)
pairs and a partial output projection (contraction over its 4 heads'
256 hidden dims).  The host sums the two half-hidden partials per batch
and adds the bias.

Math note: the reference's stabilized-softmax + detached-expsum
reweighting collapses to the unstabilized form
    out[p] = (sum_d exp(S_d) @ V_d  scattered to p) / (sum_d rowsum exp(S_d))
which is what the kernel computes (scores ~ N(0,1), no overflow risk).

Device layout per (b,h), per dilation branch: the host packs ONE blob
[128, W] per branch holding, in SBUF layout:
  - Q^T [64, Ld] duplicated onto both partition halves (matmul rhs for
    both PE row groups),
  - K^T k-tiles parity-split: even k-tiles on partitions 0-63, odd on
    64-127 (so consecutive K=64 QK matmuls land on different PE row
    groups and run concurrently),
  - V k-tile slabs [128, 65] with a ones column (PV matmul with M=65
    gives the exp row-sum on psum row 64 for free).
One DMA per branch.  S^T = matmul(lhsT=K^T[64,128], rhs=Q^T[64,512]) to
PSUM; exp on ScalarE in 3-k-tile batches (N=1536 ACTIVATEs cut the
~470ns per-call overhead on the bottleneck engine by a third); PV
accumulates over k-tiles into a [65, 512] psum window; DVE scatter-adds
windows into per-head accumulators [65, 4096]; 1/w via a [1,L]->[128,32]
DMA reshape + custom-DVE fast reciprocal (128 lanes instead of 1) + K=1
ones-matmul partition broadcast; o_proj as 2 accumulating K=128 matmuls
per M-tile against head-pair-stacked Wo^T slices (odd heads moved to
partitions 64-127 by a SBUF->SBUF DMA), psum->sbuf copies alternating
ScalarE/VectorE.

Matmul operands are bf16 (fp32 matmuls run as two PE passes on trn2);
psum accumulation and the softmax combine stay fp32.  The PE stream is
software-pipelined (QK/exp of group i+1 issued before PV of group i) so
the in-order PE queue never head-of-line blocks on the ScalarE exp.
"""

import ml_dtypes
import numpy as np

BF16_NP = ml_dtypes.bfloat16

B, H, L, HD = 4, 8, 4096, 64
HIDDEN = H * HD
DILS = (1, 2, 4, 8)
BLOCK = 1024
PB = 4  # (b,h) pairs per core
NCORES = 8
LDS = [L // d for d in DILS]  # 4096, 2048, 1024, 512
NKTS = [ld // 128 for ld in LDS]  # 32, 16, 8, 4
# blob widths per branch: Q dup (Ld) + K parity-split (Ld/2) + V slabs (nkt*65)
WS = [ld + ld // 2 + nkt * 65 for ld, nkt in zip(LDS, NKTS)]
BOFFS = [sum(WS[:i]) for i in range(len(WS))]
WSUM = sum(WS)
QCH = 512  # q-chunk (strided-domain positions) per psum window

_PROGRAM = None


def build_program():
    """Build the (SPMD, identical on all cores) Bass program."""
    from contextlib import ExitStack

    import concourse.tile as tile
    from concourse import bacc, mybir

    F32 = mybir.dt.float32
    BF16 = mybir.dt.bfloat16
    nc = bacc.Bacc("TRN2", target_bir_lowering=False, debug=False)

    blob_d = nc.dram_tensor("blob", [PB, 128, WSUM], BF16, kind="ExternalInput")
    # head-pair-stacked Wo^T slices: wot[p, k<64] = head 2p dim k,
    # wot[p, k>=64] = head 2p+1 dim k-64 (K=128 o_proj contraction)
    wot_d = nc.dram_tensor("wot", [PB // 2, 128, HIDDEN], BF16, kind="ExternalInput")
    out_d = nc.dram_tensor("out", [L, HIDDEN], F32, kind="ExternalOutput")

    with tile.TileContext(nc) as tc, ExitStack() as ctx:
        consts = ctx.enter_context(tc.tile_pool(name="consts", bufs=1))
        br_pool = ctx.enter_context(tc.tile_pool(name="br", bufs=1))
        e_pool = ctx.enter_context(tc.tile_pool(name="ep", bufs=5))
        acc_pool = ctx.enter_context(tc.tile_pool(name="accp", bufs=1))
        io_pool = ctx.enter_context(tc.tile_pool(name="iop", bufs=2))
        # st tiles hold 3 k-tiles of scores (one ACTIVATE of N=1536 instead
        # of 1.5 of N=1024 -- the ~470ns per-call ScalarE overhead is the
        # top non-stream cost on the bottleneck engine).  2 bufs x 3 banks
        # keeps the same 6 in-flight score k-tiles as 3 bufs x 2 banks.
        st_psum = ctx.enter_context(tc.tile_pool(name="stp", bufs=2, space="PSUM"))
        pv_psum = ctx.enter_context(tc.tile_pool(name="pvp", bufs=2, space="PSUM"))

        zero_bias = consts.tile([128, 1], F32, tag="zb")
        nc.vector.memset(zero_bias, 0.0)
        ones_row = consts.tile([1, 64], BF16, tag="ones_row")
        nc.vector.memset(ones_row, 1.0)
        # fp32 ones at partition 64: lhsT of the K=1 broadcast matmul whose
        # rhs is the exp-sum row acc[64:65] read in place (fp32, 2-pass PE)
        ones64 = consts.tile([128, 64], F32, tag="ones64")
        nc.vector.memset(ones64, 1.0)

        wot_sb = consts.tile([128, PB // 2, HIDDEN], BF16, tag="wot")
        nc.sync.dma_start(out=wot_sb, in_=wot_d.rearrange("j r c -> r j c"))

        acc_tiles = [
            acc_pool.tile([65, L], F32, tag=f"acc{j}", bufs=1, name=f"acc{j}")
            for j in range(PB)
        ]
        # o_proj lhsT: head pair p stacked on partitions [0:64] / [64:128]
        oacc_tiles = [
            acc_pool.tile([128, L], BF16, tag=f"oacc{p}", bufs=1, name=f"oacc{p}")
            for p in range(PB // 2)
        ]

        bt_all = {}

        def get_bt(j, di):
            if (j, di) not in bt_all:
                bufs = 1 if di <= 1 else 2
                bt = br_pool.tile(
                    [128, WS[di]], BF16, tag=f"b{di}", bufs=bufs, name=f"bt{di}"
                )
                nc.sync.dma_start(
                    out=bt, in_=blob_d[j, :, BOFFS[di] : BOFFS[di] + WS[di]]
                )
                bt_all[(j, di)] = bt
            return bt_all[(j, di)]

        for j in range(PB):
            acc = acc_tiles[j]
            # d=1 is processed LAST (acc zeroed by the otherwise-idle
            # GpSimd engine; every branch combine is then an add), so each
            # d=1 window close finalizes its positions' exp-sum row and
            # the normalize runs per-window, overlapped with later blocks,
            # instead of as a serial per-head tail.
            nc.gpsimd.memset(acc, 0.0)
            half = j % 2
            if half == 0:
                odst = oacc_tiles[j // 2]
            else:
                odst = io_pool.tile([64, L], BF16, tag="oscr", bufs=2)

            # Build the flat job list: one job per (branch, window, k-group).
            jobs = []
            for di in (1, 2, 3, 0):
                d = DILS[di]
                Ld = LDS[di]
                bs = min(BLOCK, Ld)
                nblk = Ld // bs
                nkt_blk = bs // 128
                for blk in range(nblk):
                    for qc in range(bs // QCH):
                        q0 = blk * bs + qc * QCH
                        kts = list(range(nkt_blk))
                        groups = [kts[x : x + 3] for x in range(0, nkt_blk, 3)]
                        for gi, g in enumerate(groups):
                            jobs.append(
                                dict(
                                    di=di,
                                    d=d,
                                    blk=blk,
                                    nkt_blk=nkt_blk,
                                    q0=q0,
                                    g=g,
                                    first=(gi == 0),
                                    last=(gi == len(groups) - 1),
                                    done0=sum(len(x) for x in groups[:gi]),
                                )
                            )

            # prefetch the first branches (d2 runs first now)
            get_bt(j, 1)
            get_bt(j, 2)

            def emit_qk_exp(job):
                """QK matmuls for the group -> exp to a bf16 E tile."""
                di, q0, g = job["di"], job["q0"], job["g"]
                Ld = LDS[di]
                kbase = Ld
                bt = get_bt(j, di)
                gs = len(g)
                st = st_psum.tile([128, 3, QCH], F32, tag="st", name="st")
                for i, kt in enumerate(g):
                    tg = job["blk"] * job["nkt_blk"] + kt
                    half = tg % 2
                    k0 = kbase + (tg // 2) * 128
                    nc.tensor.matmul(
                        st[:, i, :],
                        bt[half * 64 : (half + 1) * 64, k0 : k0 + 128],
                        bt[half * 64 : (half + 1) * 64, q0 : q0 + QCH],
                        start=True,
                        stop=True,
                    )
                et = e_pool.tile([128, 3, QCH], BF16, tag="et", name="et")
                nc.scalar.activation(
                    et[:, 0:gs, :],
                    st[:, 0:gs, :],
                    mybir.ActivationFunctionType.Exp,
                    bias=zero_bias,
                    scale=0.125,
                )
                job["et"] = et

            def emit_pv(job):
                """PV accumulation for the group; combine if window done."""
                di, d = job["di"], job["d"]
                Ld = LDS[di]
                vbase = Ld + Ld // 2
                bt = get_bt(j, di)
                et = job["et"]
                pv = job["pv"]
                done = job["done0"]
                for i, kt in enumerate(job["g"]):
                    tg = job["blk"] * job["nkt_blk"] + kt
                    nc.tensor.matmul(
                        pv[0:65, :],
                        bt[:, vbase + tg * 65 : vbase + tg * 65 + 65],
                        et[:, i, :],
                        start=(done == 0),
                        stop=(done == job["nkt_blk"] - 1),
                        skip_group_check=True,
                    )
                    done += 1
                if job["last"]:
                    p0 = job["q0"] * d
                    dst = acc[:, p0 : p0 + QCH * d : d]
                    nc.vector.tensor_add(out=dst, in0=dst, in1=pv[0:65, :])
                    if di == 0:
                        # d=1 is the final branch for these positions: the
                        # exp-sum row acc[64] is complete.  Normalize this
                        # window in place, reusing the just-closed pv tile's
                        # rows 0..63 (same pool generation, disjoint from
                        # nothing live): broadcast w with a K=1 ones-matmul
                        # reading acc[64:65] as an fp32 rhs, fast-reciprocal
                        # it into SBUF, multiply.  No DMA roundtrip, fully
                        # overlapped with the remaining d=1 blocks.
                        ws = slice(p0, p0 + QCH)
                        nc.tensor.matmul(
                            pv[0:64, :],
                            ones64[64:65, :],
                            acc[64:65, ws],
                            start=True,
                            stop=True,
                            skip_group_check=True,
                        )
                        bcr = io_pool.tile(
                            [64, QCH], F32, tag="bcr", bufs=2, name="bcr"
                        )
                        nc.vector.reciprocal_approx_fast(
                            out=bcr, in_=pv[0:64, :]
                        )
                        nc.vector.tensor_mul(
                            out=odst[0:64, ws], in0=acc[0:64, ws], in1=bcr
                        )
                        if half == 1:
                            # DVE lanes are partition-locked; a SBUF->SBUF
                            # DMA moves the odd head onto partitions 64..127
                            # of the pair-stacked o_proj lhsT
                            nc.sync.dma_start(
                                out=oacc_tiles[j // 2][64:128, ws],
                                in_=odst[:, ws],
                            )

            # software pipeline, depth 2: QK/exp of job i, then PV of job
            # i-2, so the in-order PE queue never blocks on the ACT exp.
            from collections import deque

            pending = deque()
            cur_pv = None
            for idx, job in enumerate(jobs):
                if job["first"]:
                    cur_pv = pv_psum.tile([128, QCH], F32, tag="pv", name="pv")
                job["pv"] = cur_pv
                if idx > 0 and job["di"] != jobs[idx - 1]["di"]:
                    # prefetch upcoming branch blobs (d4 phase is long
                    # enough to cover the 2.1MB d=1 blob DMA)
                    if job["di"] == 2:
                        get_bt(j, 3)
                        get_bt(j, 0)
                emit_qk_exp(job)
                pending.append(job)
                if len(pending) > 3:
                    emit_pv(pending.popleft())
            while pending:
                emit_pv(pending.popleft())

        # partial o_proj: out[p, :] = sum_pairs oaccP[:, p]^T @ wot_p (K=128)
        for mt in range(L // 128):
            po = pv_psum.tile([128, HIDDEN], F32, tag="pv", name="po")
            for p in range(PB // 2):
                nc.tensor.matmul(
                    po,
                    oacc_tiles[p][:, mt * 128 : (mt + 1) * 128],
                    wot_sb[:, p, :],
                    start=(p == 0),
                    stop=(p == PB // 2 - 1),
                    skip_group_check=True,
                )
            ot = io_pool.tile([128, HIDDEN], F32, tag="ot")
            if mt % 2 == 0:
                nc.scalar.copy(out=ot, in_=po)
            else:
                nc.vector.tensor_copy(out=ot, in_=po)
            nc.sync.dma_start(out=out_d[mt * 128 : (mt + 1) * 128, :], in_=ot)

    nc.compile()
    return nc


def get_program():
    global _PROGRAM
    if _PROGRAM is None:
        _PROGRAM = build_program()
    return _PROGRAM


def _branch_blob(qT, kT, vv, di):
    """Pack one dilation branch into the [128, W] SBUF-layout blob.

    qT, kT: [64, Ld] transposed Q/K for this branch; vv: [Ld, 65] V plus
    ones column."""
    Ld, nkt = LDS[di], NKTS[di]
    q_part = np.concatenate([qT, qT], axis=0)  # [128, Ld]
    k3 = kT.reshape(64, nkt, 128)
    k_part = np.concatenate(
        [
            k3[:, 0::2, :].reshape(64, -1),
            k3[:, 1::2, :].reshape(64, -1),
        ],
        axis=0,
    )  # [128, Ld/2]
    v_part = vv.reshape(nkt, 128, 65).transpose(1, 0, 2).reshape(128, nkt * 65)
    return np.concatenate([q_part, k_part, v_part], axis=1)


def make_in_maps(query_states, key_states, value_states, Wo):
    q = np.asarray(query_states, dtype=np.float32)
    k = np.asarray(key_states, dtype=np.float32)
    v = np.asarray(value_states, dtype=np.float32)
    Wo = np.asarray(Wo, dtype=np.float32)

    in_maps = []
    for c in range(NCORES):
        b, hs = c // 2, (c % 2) * PB
        blob = np.empty((PB, 128, WSUM), BF16_NP)
        wot = np.empty((PB // 2, 128, HIDDEN), BF16_NP)
        for j in range(PB):
            h = hs + j
            for di, d in enumerate(DILS):
                Ld = LDS[di]
                vv = np.empty((Ld, 65), np.float32)
                vv[:, 0:64] = v[b, h, ::d, :]
                vv[:, 64] = 1.0
                blob[j, :, BOFFS[di] : BOFFS[di] + WS[di]] = _branch_blob(
                    np.ascontiguousarray(q[b, h, ::d, :].T),
                    np.ascontiguousarray(k[b, h, ::d, :].T),
                    vv,
                    di,
                )
            # head-pair-stacked o_proj weights (K=128 contraction)
            wot[j // 2, (j % 2) * 64 : (j % 2 + 1) * 64, :] = Wo[
                :, h * 64 : (h + 1) * 64
            ].T
        in_maps.append({"blob": blob, "wot": wot})
    return in_maps


def combine_outputs(results, bo):
    bo = np.asarray(bo, dtype=np.float32)
    out = np.empty((B, L, HIDDEN), np.float32)
    for b in range(B):
        out[b] = results[2 * b]["out"] + results[2 * b + 1]["out"] + bo
    return out


def kernel(
    query_states,
    key_states,
    value_states,
    Wo,
    bo,
    _trace=False,
    _tmpdir=None,
    _results=[None],
):
    from concourse.bass_utils import run_bass_kernel_spmd

    nc = get_program()
    in_maps = make_in_maps(query_states, key_states, value_states, Wo)
    res = run_bass_kernel_spmd(
        nc, in_maps, list(range(NCORES)), trace=_trace, tmpdir=_tmpdir
    )
    _results[0] = res
    return combine_outputs(res.results, bo)



# revision 55
# speedup vs baseline: 1.1450x; 1.0385x over previous
"""Dilated block attention + output projection on 8 trn2 cores.

Sharding: core c handles batch b = c//2 and heads h = 4*(c%2) .. +3.
Each core computes the full dilated-attention combine for its 4 (b,h)
pairs and a partial output projection (contraction over its 4 heads'
256 hidden dims).  The host sums the two half-hidden partials per batch
and adds the bias.

Math note: the reference's stabilized-softmax + detached-expsum
reweighting collapses to the unstabilized form
    out[p] = (sum_d exp(S_d) @ V_d  scattered to p) / (sum_d rowsum exp(S_d))
which is what the kernel computes (scores ~ N(0,1), no overflow risk).

Device layout per (b,h), per dilation branch: the host packs ONE blob
[128, W] per branch holding, in SBUF layout:
  - Q^T [64, Ld] duplicated onto both partition halves (matmul rhs for
    both PE row groups),
  - K^T k-tiles parity-split: even k-tiles on partitions 0-63, odd on
    64-127 (so consecutive K=64 QK matmuls land on different PE row
    groups and run concurrently),
  - V k-tile slabs [128, 65] with a ones column (PV matmul with M=65
    gives the exp row-sum on psum row 64 for free).
One DMA per branch.  S^T = matmul(lhsT=K^T[64,128], rhs=Q^T[64,512]) to
PSUM; exp on ScalarE in 3-k-tile batches (N=1536 ACTIVATEs cut the
~470ns per-call overhead on the bottleneck engine by a third); PV
accumulates over k-tiles into a [65, 512] psum window; DVE scatter-adds
windows into per-head accumulators [65, 4096]; 1/w via a [1,L]->[128,32]
DMA reshape + custom-DVE fast reciprocal (128 lanes instead of 1) + K=1
ones-matmul partition broadcast; o_proj as 2 accumulating K=128 matmuls
per M-tile against head-pair-stacked Wo^T slices (odd heads moved to
partitions 64-127 by a SBUF->SBUF DMA), psum->sbuf copies alternating
ScalarE/VectorE.

Matmul operands are bf16 (fp32 matmuls run as two PE passes on trn2);
psum accumulation and the softmax combine stay fp32.  The PE stream is
software-pipelined (QK/exp of group i+1 issued before PV of group i) so
the in-order PE queue never head-of-line blocks on the ScalarE exp.
"""

import ml_dtypes
import numpy as np

BF16_NP = ml_dtypes.bfloat16

B, H, L, HD = 4, 8, 4096, 64
HIDDEN = H * HD
DILS = (1, 2, 4, 8)
BLOCK = 1024
PB = 4  # (b,h) pairs per core
NCORES = 8
LDS = [L // d for d in DILS]  # 4096, 2048, 1024, 512
NKTS = [ld // 128 for ld in LDS]  # 32, 16, 8, 4
# blob widths per branch: Q dup (Ld) + K parity-split (Ld/2) + V slabs (nkt*65)
WS = [ld + ld // 2 + nkt * 65 for ld, nkt in zip(LDS, NKTS)]
BOFFS = [sum(WS[:i]) for i in range(len(WS))]
WSUM = sum(WS)
QCH = 512  # q-chunk (strided-domain positions) per psum window

# Schraudolph fast-exp on the DVE for alternating d2/d4/d8 score tiles
# (the DVE-light phase): bf16 bits of exp(s/8) ~= round(s*128*log2(e)/8
# + 128*(127-C)) written as int16 via AP bitcast.  ~23% of the exp work
# moves off the bottleneck ScalarE; the d=1 phase (DVE busy with the
# combine+normalize) stays on ScalarE.
SCH_C = 0.0437
SCH_A = float(128.0 * np.log2(np.e) / 8.0)
SCH_B = float(128.0 * (127.0 - SCH_C))

_PROGRAM = None


def build_program():
    """Build the (SPMD, identical on all cores) Bass program."""
    from contextlib import ExitStack

    import concourse.tile as tile
    from concourse import bacc, mybir

    F32 = mybir.dt.float32
    BF16 = mybir.dt.bfloat16
    nc = bacc.Bacc("TRN2", target_bir_lowering=False, debug=False)

    blob_d = nc.dram_tensor("blob", [PB, 128, WSUM], BF16, kind="ExternalInput")
    # head-pair-stacked Wo^T slices: wot[p, k<64] = head 2p dim k,
    # wot[p, k>=64] = head 2p+1 dim k-64 (K=128 o_proj contraction)
    wot_d = nc.dram_tensor("wot", [PB // 2, 128, HIDDEN], BF16, kind="ExternalInput")
    out_d = nc.dram_tensor("out", [L, HIDDEN], F32, kind="ExternalOutput")

    with tile.TileContext(nc) as tc, ExitStack() as ctx:
        consts = ctx.enter_context(tc.tile_pool(name="consts", bufs=1))
        br_pool = ctx.enter_context(tc.tile_pool(name="br", bufs=1))
        e_pool = ctx.enter_context(tc.tile_pool(name="ep", bufs=5))
        acc_pool = ctx.enter_context(tc.tile_pool(name="accp", bufs=1))
        io_pool = ctx.enter_context(tc.tile_pool(name="iop", bufs=2))
        # st tiles hold 3 k-tiles of scores (one ACTIVATE of N=1536 instead
        # of 1.5 of N=1024 -- the ~470ns per-call ScalarE overhead is the
        # top non-stream cost on the bottleneck engine).  2 bufs x 3 banks
        # keeps the same 6 in-flight score k-tiles as 3 bufs x 2 banks.
        st_psum = ctx.enter_context(tc.tile_pool(name="stp", bufs=2, space="PSUM"))
        pv_psum = ctx.enter_context(tc.tile_pool(name="pvp", bufs=2, space="PSUM"))

        zero_bias = consts.tile([128, 1], F32, tag="zb")
        nc.vector.memset(zero_bias, 0.0)
        ones_row = consts.tile([1, 64], BF16, tag="ones_row")
        nc.vector.memset(ones_row, 1.0)
        # fp32 ones at partition 64: lhsT of the K=1 broadcast matmul whose
        # rhs is the exp-sum row acc[64:65] read in place (fp32, 2-pass PE)
        ones64 = consts.tile([128, 64], F32, tag="ones64")
        nc.vector.memset(ones64, 1.0)

        wot_sb = consts.tile([128, PB // 2, HIDDEN], BF16, tag="wot")
        nc.sync.dma_start(out=wot_sb, in_=wot_d.rearrange("j r c -> r j c"))

        acc_tiles = [
            acc_pool.tile([65, L], F32, tag=f"acc{j}", bufs=1, name=f"acc{j}")
            for j in range(PB)
        ]
        # o_proj lhsT: head pair p stacked on partitions [0:64] / [64:128]
        oacc_tiles = [
            acc_pool.tile([128, L], BF16, tag=f"oacc{p}", bufs=1, name=f"oacc{p}")
            for p in range(PB // 2)
        ]

        bt_all = {}

        def get_bt(j, di):
            if (j, di) not in bt_all:
                bufs = 1 if di <= 1 else 2
                bt = br_pool.tile(
                    [128, WS[di]], BF16, tag=f"b{di}", bufs=bufs, name=f"bt{di}"
                )
                nc.sync.dma_start(
                    out=bt, in_=blob_d[j, :, BOFFS[di] : BOFFS[di] + WS[di]]
                )
                bt_all[(j, di)] = bt
            return bt_all[(j, di)]

        for j in range(PB):
            acc = acc_tiles[j]
            # d=1 is processed LAST (acc zeroed by the otherwise-idle
            # GpSimd engine; every branch combine is then an add), so each
            # d=1 window close finalizes its positions' exp-sum row and
            # the normalize runs per-window, overlapped with later blocks,
            # instead of as a serial per-head tail.
            nc.gpsimd.memset(acc, 0.0)
            half = j % 2
            if half == 0:
                odst = oacc_tiles[j // 2]
            else:
                odst = io_pool.tile([64, L], BF16, tag="oscr", bufs=2)

            # Build the flat job list: one job per (branch, window, k-group).
            jobs = []
            for di in (3, 2, 1, 0):
                d = DILS[di]
                Ld = LDS[di]
                bs = min(BLOCK, Ld)
                nblk = Ld // bs
                nkt_blk = bs // 128
                for blk in range(nblk):
                    for qc in range(bs // QCH):
                        q0 = blk * bs + qc * QCH
                        kts = list(range(nkt_blk))
                        groups = [kts[x : x + 3] for x in range(0, nkt_blk, 3)]
                        for gi, g in enumerate(groups):
                            jobs.append(
                                dict(
                                    di=di,
                                    d=d,
                                    blk=blk,
                                    nkt_blk=nkt_blk,
                                    q0=q0,
                                    g=g,
                                    first=(gi == 0),
                                    last=(gi == len(groups) - 1),
                                    done0=sum(len(x) for x in groups[:gi]),
                                    dve_exp=(di != 0 and len(jobs) % 2 == 1),
                                )
                            )

            # prefetch the first branches (d8 runs first: smallest blob)
            get_bt(j, 3)
            get_bt(j, 2)

            def emit_qk_exp(job):
                """QK matmuls for the group -> exp to a bf16 E tile."""
                di, q0, g = job["di"], job["q0"], job["g"]
                Ld = LDS[di]
                kbase = Ld
                bt = get_bt(j, di)
                gs = len(g)
                st = st_psum.tile([128, 3, QCH], F32, tag="st", name="st")
                for i, kt in enumerate(g):
                    tg = job["blk"] * job["nkt_blk"] + kt
                    half = tg % 2
                    k0 = kbase + (tg // 2) * 128
                    nc.tensor.matmul(
                        st[:, i, :],
                        bt[half * 64 : (half + 1) * 64, k0 : k0 + 128],
                        bt[half * 64 : (half + 1) * 64, q0 : q0 + QCH],
                        start=True,
                        stop=True,
                    )
                et = e_pool.tile([128, 3, QCH], BF16, tag="et", name="et")
                if job["dve_exp"]:
                    nc.vector.tensor_scalar(
                        et[:, 0:gs, :].bitcast(mybir.dt.int16),
                        st[:, 0:gs, :],
                        SCH_A,
                        SCH_B,
                        mybir.AluOpType.mult,
                        mybir.AluOpType.add,
                    )
                else:
                    nc.scalar.activation(
                        et[:, 0:gs, :],
                        st[:, 0:gs, :],
                        mybir.ActivationFunctionType.Exp,
                        bias=zero_bias,
                        scale=0.125,
                    )
                job["et"] = et

            def emit_pv(job):
                """PV accumulation for the group; combine if window done."""
                di, d = job["di"], job["d"]
                Ld = LDS[di]
                vbase = Ld + Ld // 2
                bt = get_bt(j, di)
                et = job["et"]
                pv = job["pv"]
                done = job["done0"]
                for i, kt in enumerate(job["g"]):
                    tg = job["blk"] * job["nkt_blk"] + kt
                    nc.tensor.matmul(
                        pv[0:65, :],
                        bt[:, vbase + tg * 65 : vbase + tg * 65 + 65],
                        et[:, i, :],
                        start=(done == 0),
                        stop=(done == job["nkt_blk"] - 1),
                        skip_group_check=True,
                    )
                    done += 1
                if job["last"]:
                    p0 = job["q0"] * d
                    dst = acc[:, p0 : p0 + QCH * d : d]
                    nc.vector.tensor_add(out=dst, in0=dst, in1=pv[0:65, :])
                    if di == 0:
                        # d=1 is the final branch for these positions: the
                        # exp-sum row acc[64] is complete.  Normalize this
                        # window in place, reusing the just-closed pv tile's
                        # rows 0..63 (same pool generation, disjoint from
                        # nothing live): broadcast w with a K=1 ones-matmul
                        # reading acc[64:65] as an fp32 rhs, fast-reciprocal
                        # it into SBUF, multiply.  No DMA roundtrip, fully
                        # overlapped with the remaining d=1 blocks.
                        ws = slice(p0, p0 + QCH)
                        nc.tensor.matmul(
                            pv[0:64, :],
                            ones64[64:65, :],
                            acc[64:65, ws],
                            start=True,
                            stop=True,
                            skip_group_check=True,
                        )
                        bcr = io_pool.tile(
                            [64, QCH], F32, tag="bcr", bufs=2, name="bcr"
                        )
                        nc.vector.reciprocal_approx_fast(
                            out=bcr, in_=pv[0:64, :]
                        )
                        nc.vector.tensor_mul(
                            out=odst[0:64, ws], in0=acc[0:64, ws], in1=bcr
                        )
                        if half == 1:
                            # DVE lanes are partition-locked; a SBUF->SBUF
                            # DMA moves the odd head onto partitions 64..127
                            # of the pair-stacked o_proj lhsT
                            nc.sync.dma_start(
                                out=oacc_tiles[j // 2][64:128, ws],
                                in_=odst[:, ws],
                            )

            # software pipeline, depth 2: QK/exp of job i, then PV of job
            # i-2, so the in-order PE queue never blocks on the ACT exp.
            from collections import deque

            pending = deque()
            cur_pv = None
            for idx, job in enumerate(jobs):
                if job["first"]:
                    cur_pv = pv_psum.tile([128, QCH], F32, tag="pv", name="pv")
                job["pv"] = cur_pv
                if idx > 0 and job["di"] != jobs[idx - 1]["di"]:
                    # prefetch upcoming branch blobs one phase ahead (the
                    # d2 phase is long enough to cover the 2.1MB d=1 blob)
                    if job["di"] == 2:
                        get_bt(j, 1)
                    elif job["di"] == 1:
                        get_bt(j, 0)
                emit_qk_exp(job)
                pending.append(job)
                if len(pending) > 3:
                    emit_pv(pending.popleft())
            while pending:
                emit_pv(pending.popleft())

        # partial o_proj: out[p, :] = sum_pairs oaccP[:, p]^T @ wot_p (K=128)
        for mt in range(L // 128):
            po = pv_psum.tile([128, HIDDEN], F32, tag="pv", name="po")
            for p in range(PB // 2):
                nc.tensor.matmul(
                    po,
                    oacc_tiles[p][:, mt * 128 : (mt + 1) * 128],
                    wot_sb[:, p, :],
                    start=(p == 0),
                    stop=(p == PB // 2 - 1),
                    skip_group_check=True,
                )
            ot = io_pool.tile([128, HIDDEN], F32, tag="ot")
            if mt % 2 == 0:
                nc.scalar.copy(out=ot, in_=po)
            else:
                nc.vector.tensor_copy(out=ot, in_=po)
            nc.sync.dma_start(out=out_d[mt * 128 : (mt + 1) * 128, :], in_=ot)

    nc.compile()
    return nc


def get_program():
    global _PROGRAM
    if _PROGRAM is None:
        _PROGRAM = build_program()
    return _PROGRAM


def _branch_blob(qT, kT, vv, di):
    """Pack one dilation branch into the [128, W] SBUF-layout blob.

    qT, kT: [64, Ld] transposed Q/K for this branch; vv: [Ld, 65] V plus
    ones column."""
    Ld, nkt = LDS[di], NKTS[di]
    q_part = np.concatenate([qT, qT], axis=0)  # [128, Ld]
    k3 = kT.reshape(64, nkt, 128)
    k_part = np.concatenate(
        [
            k3[:, 0::2, :].reshape(64, -1),
            k3[:, 1::2, :].reshape(64, -1),
        ],
        axis=0,
    )  # [128, Ld/2]
    v_part = vv.reshape(nkt, 128, 65).transpose(1, 0, 2).reshape(128, nkt * 65)
    return np.concatenate([q_part, k_part, v_part], axis=1)


def make_in_maps(query_states, key_states, value_states, Wo):
    q = np.asarray(query_states, dtype=np.float32)
    k = np.asarray(key_states, dtype=np.float32)
    v = np.asarray(value_states, dtype=np.float32)
    Wo = np.asarray(Wo, dtype=np.float32)

    in_maps = []
    for c in range(NCORES):
        b, hs = c // 2, (c % 2) * PB
        blob = np.empty((PB, 128, WSUM), BF16_NP)
        wot = np.empty((PB // 2, 128, HIDDEN), BF16_NP)
        for j in range(PB):
            h = hs + j
            for di, d in enumerate(DILS):
                Ld = LDS[di]
                vv = np.empty((Ld, 65), np.float32)
                vv[:, 0:64] = v[b, h, ::d, :]
                vv[:, 64] = 1.0
                blob[j, :, BOFFS[di] : BOFFS[di] + WS[di]] = _branch_blob(
                    np.ascontiguousarray(q[b, h, ::d, :].T),
                    np.ascontiguousarray(k[b, h, ::d, :].T),
                    vv,
                    di,
                )
            # head-pair-stacked o_proj weights (K=128 contraction)
            wot[j // 2, (j % 2) * 64 : (j % 2 + 1) * 64, :] = Wo[
                :, h * 64 : (h + 1) * 64
            ].T
        in_maps.append({"blob": blob, "wot": wot})
    return in_maps


def combine_outputs(results, bo):
    bo = np.asarray(bo, dtype=np.float32)
    out = np.empty((B, L, HIDDEN), np.float32)
    for b in range(B):
        out[b] = results[2 * b]["out"] + results[2 * b + 1]["out"] + bo
    return out


def kernel(
    query_states,
    key_states,
    value_states,
    Wo,
    bo,
    _trace=False,
    _tmpdir=None,
    _results=[None],
):
    from concourse.bass_utils import run_bass_kernel_spmd

    nc = get_program()
    in_maps = make_in_maps(query_states, key_states, value_states, Wo)
    res = run_bass_kernel_spmd(
        nc, in_maps, list(range(NCORES)), trace=_trace, tmpdir=_tmpdir
    )
    _results[0] = res
    return combine_outputs(res.results, bo)

